# revision 14
# baseline (speedup 1.0000x reference)
"""Experts-choose MoE MLP kernel for 8 TRN2 NeuronCores — bf16 pipeline v4.

Sharding: core = 2*b + half handles batch row b and experts
[4*half, 4*half+4). Each core emits one bf16 partial output per local
expert (4 DRAM buffers); host sums them per batch row in fp32 and adds
the shared-bias term total_gate (x) b2 (moved off-device entirely).

Host prep: per (core, expert) the C=1024 slots are deduplicated to
unique token ids with summed gates (sorted ascending).  Duplicates are
gone BEFORE the device sees them, so the per-chunk merge matmul uses a
plain diagonal gate matrix and scatters are PLAIN writes (no CCE
read-modify-write): within one expert every chunk writes disjoint token
rows of that expert's private output buffer.  Pad slots (beyond the
unique count) gather token 0 with gate 0 and scatter to an OOB index
that the bounds check drops.

Per 128-token chunk on device (5-stage software pipeline):
  A: dma_gather(transpose=True) — gathers x rows from HBM and writes
     them TRANSPOSED (selT layout) in one SWDGE op; 2 chunks per op.
     Gathers alternate SWDGE queues 1/2 so they never queue behind
     scatters (queue 0) — the v3 bottleneck (42us PE stalls).
  C: GEMM1 (k=D bf16, b1 folded as k=1 matmul) + exact Gelu -> h bf16
  D: gate/transpose matmul h^T @ diag(gate) -> ht2 bf16
  E: GEMM2 (k=H bf16, no bias) -> y bf16 (PSUM->SBUF casts split
     across Vector and Scalar engines)
  F: plain indirect scatter (no compute op) into the expert's own
     output buffer; chains per buffer are only 8 deep.
Weight/const loads go on the scalar HWDGE ring; tiny index loads go
first on the sync ring.  Output buffers are pre-zeroed by the runtime;
untouched tokens stay 0.
"""

import threading

import numpy as np
import ml_dtypes

import concourse.mybir as mybir
import concourse.tile as tile
from concourse import bacc
from concourse.bass import IndirectOffsetOnAxis
from concourse.bass_utils import run_bass_kernel_spmd

P = 128
B, T, D, E, C = 4, 4096, 2048, 8, 1024
H = 256
E_LOC = 4
NCB = C // P          # 8 chunks per expert
NCP = NCB // 2        # 4 steps per expert (2 chunks per step)
NCH = E_LOC * NCB     # 32 chunks per core
N_CORES = 8
OOB = 1 << 28         # scatter index for dropped pad slots

BF16 = mybir.dt.bfloat16
F32 = mybir.dt.float32
I32 = mybir.dt.int32
I16 = mybir.dt.int16
AF = mybir.ActivationFunctionType

GIDX_COLS = E_LOC * C // 16   # 256 int16 columns (wrapped in 16 partitions)
HYB = 2                       # startup steps served by host-pregathered selT

ACT_FUNC = AF.Gelu  # sim_debug overrides (interp lacks Gelu)


def build_kernel():
    nc = bacc.Bacc("TRN2", target_bir_lowering=False, debug=False)

    x = nc.dram_tensor("x", [T, D], BF16, kind="ExternalInput").ap()
    w1t = nc.dram_tensor("w1t", [E_LOC, D, H], BF16, kind="ExternalInput").ap()
    w2t = nc.dram_tensor("w2t", [E_LOC, H, D], BF16, kind="ExternalInput").ap()
    b1 = nc.dram_tensor("b1", [E_LOC, H], BF16, kind="ExternalInput").ap()
    gidx = nc.dram_tensor("gidx", [P, GIDX_COLS], I16,
                          kind="ExternalInput").ap()
    selt0_d = nc.dram_tensor("selt0", [P, D // P, 2 * P * HYB], BF16,
                             kind="ExternalInput").ap()
    sgl_d = nc.dram_tensor("sgl", [P, NCH, P], BF16,
                           kind="ExternalInput").ap()
    ones_d = nc.dram_tensor("ones", [1, P], BF16, kind="ExternalInput").ap()
    y_dram = nc.dram_tensor("y_all", [NCH * P, D], BF16,
                            kind="ExternalOutput").ap()

    steps = [(e, cp) for e in range(E_LOC) for cp in range(NCP)]
    NS = len(steps)

    with tile.TileContext(nc) as tc:
        with (
            tc.tile_pool(name="const", bufs=1) as const,
            tc.tile_pool(name="work", bufs=2) as spool,
            tc.tile_pool(name="wts", bufs=2) as wpool,
            tc.tile_pool(name="selp", bufs=4) as selpool,
            tc.tile_pool(name="psum", bufs=2, space="PSUM") as ppool,
        ):
            # tiny index loads go FIRST on the sync ring so the first
            # gather isn't stuck behind anything
            # first-MM critical path: step-0 selT half goes first
            selt0 = const.tile([P, D // P, 2 * P * HYB], BF16, tag="selt0",
                               name="selt0")
            nc.sync.dma_start(out=selt0[:, :, :2 * P],
                              in_=selt0_d[:, :, :2 * P])
            nc.sync.dma_start(out=selt0[:, :, 2 * P:],
                              in_=selt0_d[:, :, 2 * P:])
            gidx_sb = const.tile([P, GIDX_COLS], I16, tag="gidx",
                                 name="gidx_sb")
            nc.sync.dma_start(out=gidx_sb, in_=gidx)
            ones_row = const.tile([1, P], BF16, tag="ones", name="ones_row")
            nc.scalar.dma_start(out=ones_row, in_=ones_d)
            b1_row = const.tile([1, E_LOC * H], BF16, tag="b1", name="b1_row")
            nc.scalar.dma_start(out=b1_row,
                                in_=b1.rearrange("e h -> (e h)")[None, :])

            wts = {}

            def load_expert(e):
                w1_sb = wpool.tile([P, D // P, H], BF16, tag="w1",
                                   name="w1_sb")
                nc.scalar.dma_start(
                    out=w1_sb,
                    in_=w1t[e].rearrange("(go gi) j -> gi go j", gi=P),
                )
                w2_sb = wpool.tile([P, H // P, D], BF16, tag="w2",
                                   name="w2_sb")
                nc.scalar.dma_start(
                    out=w2_sb,
                    in_=w2t[e].rearrange("(jo ji) o -> ji jo o", ji=P),
                )
                wts[e] = (w1_sb, w2_sb)

            load_expert(0)
            sgl_all = const.tile([P, NCH, P], BF16, tag="sgl", name="sgl_all")
            nc.scalar.dma_start(out=sgl_all[:, :NCB], in_=sgl_d[:, :NCB])
            nc.scalar.dma_start(out=sgl_all[:, NCB:], in_=sgl_d[:, NCB:])

            # compute-produced twins of repeatedly-read constants: consumers
            # then wait on precise DVE sems instead of coarsened per-ring
            # DMA FIFO sems (which chain them to unrelated later DMAs)
            gidx2 = const.tile([P, GIDX_COLS], I16, tag="gidx2", name="gidx2")
            nc.vector.tensor_copy(out=gidx2, in_=gidx_sb)
            ones2 = const.tile([1, P], BF16, tag="ones2", name="ones2")
            nc.vector.tensor_copy(out=ones2, in_=ones_row)
            b12 = const.tile([1, E_LOC * H], BF16, tag="b12", name="b12")
            nc.vector.tensor_copy(out=b12, in_=b1_row)
            sgl2 = const.tile([P, NCH, P], BF16, tag="sgl2", name="sgl2")
            nc.vector.tensor_copy(out=sgl2[:, :NCB], in_=sgl_all[:, :NCB])
            nc.vector.tensor_copy(out=sgl2[:, NCB:], in_=sgl_all[:, NCB:])

            st = {}  # per-step pipeline state

            def stage_a(s):
                e, cp = steps[s]
                if cp == 0 and e + 1 < E_LOC:
                    load_expert(e + 1)
                if s < HYB:
                    # served from the host-pregathered selt0 tile: the
                    # first steps must not wait for the Q7 extended-inst
                    # library that dma_gather needs (~12us install)
                    st[s] = {}
                    return
                # one transposed gather covers this step's 2 chunks:
                # selt[p, g, i] = x[gidx[step*256 + i], g*128 + p]
                selt = selpool.tile([P, D // P, 2 * P], BF16, tag="selt",
                                    name="selt", bufs=5)
                col = (e * NCP + cp) * 16
                nc.gpsimd.dma_gather(
                    selt,
                    x,
                    gidx2[:, col:col + 16],
                    2 * P,
                    2 * P,
                    D,
                    transpose=True,
                )
                st[s] = {"selt": selt}

            def stage_c(s):
                e, cp = steps[s]
                d = st[s]
                hs = []
                for hh in range(2):
                    if s < HYB:
                        base = (2 * s + hh) * P

                        def lslice(g, base=base):
                            return selt0[:, g, base:base + P]
                    else:
                        selt = d["selt"]

                        def lslice(g, selt=selt, hh=hh):
                            return selt[:, g, hh * P:(hh + 1) * P]
                    ph = ppool.tile([P, H], F32, tag="ph", name="ph",
                                    bufs=3)
                    for g in range(D // P):
                        nc.tensor.matmul(
                            out=ph,
                            lhsT=lslice(g),
                            rhs=wts[e][0][:, g, :],
                            start=(g == 0),
                            stop=False,
                        )
                    nc.tensor.matmul(
                        out=ph,
                        lhsT=ones2,
                        rhs=b12[:, e * H:(e + 1) * H],
                        start=False,
                        stop=True,
                    )
                    h_sb = spool.tile([P, H], BF16, tag=f"h{hh}", name="h_sb",
                                      bufs=3)
                    nc.scalar.activation(out=h_sb, in_=ph, func=ACT_FUNC)
                    hs.append(h_sb)
                d["hs"] = hs

            def stage_d(s):
                e, cp = steps[s]
                d = st[s]
                ht2s = []
                for hh in range(2):
                    cb = e * NCB + cp * 2 + hh
                    h_sb = d["hs"][hh]
                    ph2 = ppool.tile([P, H], F32, tag="ph", name="ph2",
                                     bufs=3)
                    for jo in range(H // P):
                        nc.tensor.matmul(
                            out=ph2[:, jo * P:(jo + 1) * P],
                            lhsT=h_sb[:, jo * P:(jo + 1) * P],
                            rhs=sgl2[:, cb, :],
                            start=True,
                            stop=True,
                        )
                    ht2 = spool.tile([P, H // P, P], BF16, tag=f"ht2{hh}",
                                     name="ht2", bufs=3)
                    nc.scalar.copy(
                        out=ht2,
                        in_=ph2.rearrange("p (jo q) -> p jo q", jo=H // P),
                    )
                    ht2s.append(ht2)
                d["ht2s"] = ht2s

            def stage_e(s):
                e, cp = steps[s]
                d = st[s]
                y_out = spool.tile([P, 2, D], BF16, tag="y", name="y_out",
                                   bufs=3)
                for hh in range(2):
                    ht2 = d["ht2s"][hh]
                    for oc in range(D // 512):
                        py = ppool.tile([P, 512], F32, tag="py", name="py",
                                        bufs=5)
                        for jo in range(H // P):
                            nc.tensor.matmul(
                                out=py,
                                lhsT=ht2[:, jo, :],
                                rhs=wts[e][1][:, jo,
                                              oc * 512:(oc + 1) * 512],
                                start=(jo == 0),
                                stop=(jo == H // P - 1),
                            )
                        dst = y_out[:, hh, oc * 512:(oc + 1) * 512]
                        last = s == NS - 1
                        if last and oc >= 2:
                            nc.scalar.copy(out=dst, in_=py)
                        else:
                            nc.vector.tensor_copy(out=dst, in_=py)
                        if last:
                            rows = slice((2 * s + hh) * P,
                                         (2 * s + hh + 1) * P)
                            nc.sync.dma_start(
                                out=y_dram[rows, oc * 512:(oc + 1) * 512],
                                in_=dst,
                            )
                d["y_out"] = y_out

            def stage_f(s):
                d = st.pop(s)
                if s == NS - 1:
                    return
                for hh in range(2):
                    rows = slice((2 * s + hh) * P, (2 * s + hh + 1) * P)
                    nc.sync.dma_start(out=y_dram[rows],
                                      in_=d["y_out"][:, hh, :])

            # software pipeline with explicit skew offsets (stage, offset)
            sched = [
                (stage_a, 0),
                (stage_c, 2),
                (stage_d, 3),
                (stage_e, 4),
                (stage_f, 5),
            ]
            max_off = max(off for _, off in sched)
            for si in range(NS + max_off):
                for fn, off in sched:
                    s = si - off
                    if 0 <= s < NS:
                        fn(s)
    nc.compile()
    return nc


_CACHE = {}
_CACHE_LOCK = threading.Lock()


def _get_nc():
    with _CACHE_LOCK:
        if "nc" not in _CACHE:
            _CACHE["nc"] = build_kernel()
        return _CACHE["nc"]


def _prep_indices(idx_e, gate_e):
    """Dedup one expert's slots: unique tokens (ascending) + summed gates.

    Returns (gid (C,) int16 gather idx, uniq (nu,) token ids,
    gate (C,) f32).  Pad slots gather token 0 with gate 0 (their y rows
    are exact zeros and the host ignores them).
    """
    uniq, inv = np.unique(idx_e, return_inverse=True)
    gsum = np.zeros(len(uniq), dtype=np.float64)
    np.add.at(gsum, inv, gate_e.astype(np.float64))
    nu = len(uniq)
    assert nu <= C
    gid = np.zeros(C, dtype=np.int16)
    gid[:nu] = uniq.astype(np.int16)
    gate = np.zeros(C, dtype=np.float32)
    gate[:nu] = gsum.astype(np.float32)
    return gid, uniq, gate


def _make_in_maps(x, W1, b1, W2, b2, expert_indices, expert_gate):
    bf16 = ml_dtypes.bfloat16
    x = np.asarray(x, dtype=np.float32)
    W1 = np.asarray(W1, dtype=np.float32)
    b1v = np.asarray(b1, dtype=bf16)
    W2 = np.asarray(W2, dtype=np.float32)
    idx_all = np.asarray(expert_indices, dtype=np.int32)
    gate_all = np.asarray(expert_gate, dtype=np.float32)

    ones_row = np.ones((1, P), dtype=bf16)

    x_bf = [np.ascontiguousarray(x[bb].astype(bf16)) for bb in range(B)]
    w1t_bf = np.ascontiguousarray(W1.transpose(0, 2, 1).astype(bf16))
    w2t_bf = np.ascontiguousarray(W2.transpose(0, 2, 1).astype(bf16))

    in_maps = []
    uniqs = []  # per core: list of E_LOC unique-token arrays (host combine)
    for core in range(N_CORES):
        bb, half = divmod(core, 2)
        es = slice(half * E_LOC, half * E_LOC + E_LOC)
        gidx_t = np.empty((P, GIDX_COLS), dtype=np.int16)
        selt0_t = None
        sgl_t = np.zeros((NCH, P, P), dtype=np.float32)
        core_uniqs = []
        for le, ge in enumerate(range(half * E_LOC, half * E_LOC + E_LOC)):
            gid, uniq, gate = _prep_indices(idx_all[bb, ge], gate_all[bb, ge])
            core_uniqs.append(uniq)
            # wrapped int16 gather layout: slot i -> [i % 16, base + i // 16]
            gidx_t[:, le * (C // 16):(le + 1) * (C // 16)] = \
                np.tile(gid.reshape(C // 16, 16).T, (P // 16, 1))
            if le == 0:
                xs = x_bf[bb][gid[:2 * HYB * P].astype(np.int64)]
                selt0_t = np.ascontiguousarray(
                    xs.reshape(2 * HYB * P, D // P, P).transpose(2, 1, 0))
            gch = gate.reshape(NCB, P)
            for cb in range(NCB):
                np.fill_diagonal(sgl_t[le * NCB + cb], gch[cb])
        uniqs.append(core_uniqs)
        in_maps.append(
            {
                "x": x_bf[bb],
                "w1t": np.ascontiguousarray(w1t_bf[es]),
                "w2t": np.ascontiguousarray(w2t_bf[es]),
                "b1": np.ascontiguousarray(b1v[es]),
                "gidx": np.ascontiguousarray(gidx_t),
                "selt0": selt0_t,
                "sgl": np.ascontiguousarray(
                    sgl_t.transpose(1, 0, 2).astype(bf16)
                ),
                "ones": ones_row,
            }
        )
    return in_maps, uniqs


def kernel(x, W1, b1, W2, b2, expert_indices, expert_gate, num_tokens, *,
           _trace=False, _trace_kwargs=None):
    assert int(num_tokens) == T
    nc = _get_nc()
    in_maps, uniqs = _make_in_maps(x, W1, b1, W2, b2, expert_indices,
                                   expert_gate)
    res = run_bass_kernel_spmd(
        nc,
        in_maps,
        core_ids=list(range(N_CORES)),
        trace=_trace,
        **(_trace_kwargs or {}),
    )
    idx_all = np.asarray(expert_indices)
    gate_all = np.asarray(expert_gate, dtype=np.float64)
    b2f = np.asarray(b2, dtype=np.float32)
    full = np.empty((B, T, D), dtype=np.float32)
    for bb in range(B):
        acc = np.zeros((T, D), dtype=np.float32)
        for half in range(2):
            core = 2 * bb + half
            y_all = np.asarray(res.results[core]["y_all"],
                               dtype=np.float32).reshape(NCH, P, D)
            for e in range(E_LOC):
                uniq = uniqs[core][e]
                y_e = y_all[e * NCB:(e + 1) * NCB].reshape(C, D)
                acc[uniq] += y_e[:len(uniq)]
        # shared output bias, weighted by each token's total gate mass
        total_gate = np.bincount(
            idx_all[bb].ravel().astype(np.int64),
            weights=gate_all[bb].ravel(),
            minlength=T,
        ).astype(np.float32)
        acc += total_gate[:, None] * b2f[None, :]
        full[bb] = acc
    if _trace:
        kernel.last_results = res
    return full


# revision 15
# speedup vs baseline: 1.0642x; 1.0642x over previous
"""Experts-choose MoE MLP kernel for 8 TRN2 NeuronCores — bf16 pipeline v10.

Sharding: core = 2*b + half handles batch row b and experts
[4*half, 4*half+4). Each core writes its 4096 gated expert-output rows
CONTIGUOUSLY to DRAM (y_all); the host places them into the (T, D)
output during unsharding (strictly less host arithmetic than summing
dense partial buffers) and adds the shared-bias term
total_gate (x) b2, which never touches the device.

Host prep: per (core, expert) the C=1024 slots are deduplicated to
unique token ids (ascending) with summed gates.  Pad slots (beyond the
unique count) gather token 0 with gate 0 — their y rows are exact
zeros and the host ignores them.  No scatter, no merge matrices.

Per 256-token step on device (4-stage software pipeline):
  A: dma_gather(transpose=True) — gathers x rows from HBM and writes
     them TRANSPOSED (selT layout [d128, d16, c256]) in one SWDGE op.
     The first HYB steps instead read a host-pregathered selt0 tile so
     the PE starts ~13us in while the Q7 extended-instruction library
     (~12us install, needed by dma_gather) loads in their shadow.
  C: GEMM1 transposed: phT[h128, c256] = sum_g w1[g,ht]^T @ selT[g]
     (weights stationary); Gelu with b1 fused as the ScalarE
     activation's PER-PARTITION bias -> h_T bf16 [h, c].  No separate
     bias matmul, no h transpose needed later.
  E: GEMM2 straight from h_T slices (lhsT = h_T[:, chunk]); the gate
     is applied during the PSUM->SBUF cast as a per-partition
     tensor_scalar_mul on DVE.  No merge matmul, no extra cast.
  F: contiguous HWDGE store of the step's 256 y rows (per chunk); the
     last step stores each 512-wide slice as it is cast for a fast
     drain.
Weight loads go on the scalar HWDGE ring (w1/w2 of expert e+1 prefetch
during expert e); index/selt0 loads go first on the sync ring.
"""

import threading

import numpy as np
import ml_dtypes

import concourse.mybir as mybir
import concourse.tile as tile
from concourse import bacc
from concourse.bass_utils import run_bass_kernel_spmd

P = 128
B, T, D, E, C = 4, 4096, 2048, 8, 1024
H = 256
E_LOC = 4
NCB = C // P          # 8 chunks per expert
NCP = NCB // 2        # 4 steps per expert (2 chunks per step)
NCH = E_LOC * NCB     # 32 chunks per core
N_CORES = 8

BF16 = mybir.dt.bfloat16
F32 = mybir.dt.float32
I16 = mybir.dt.int16
AF = mybir.ActivationFunctionType

GIDX_COLS = E_LOC * C // 16   # 256 int16 columns (wrapped in 16 partitions)
HYB = 2                       # startup steps served by host-pregathered selT

ACT_FUNC = AF.Gelu  # sim_debug overrides (interp lacks Gelu)


def build_kernel():
    nc = bacc.Bacc("TRN2", target_bir_lowering=False, debug=False)

    x = nc.dram_tensor("x", [T, D], BF16, kind="ExternalInput").ap()
    w1t = nc.dram_tensor("w1t", [E_LOC, D, H], BF16, kind="ExternalInput").ap()
    w2t = nc.dram_tensor("w2t", [E_LOC, H, D], BF16, kind="ExternalInput").ap()
    b1c_d = nc.dram_tensor("b1c", [P, 2 * E_LOC], F32,
                           kind="ExternalInput").ap()
    gates_d = nc.dram_tensor("gates", [P, NCH], F32,
                             kind="ExternalInput").ap()
    gidx = nc.dram_tensor("gidx", [P, GIDX_COLS], I16,
                          kind="ExternalInput").ap()
    selt0_d = nc.dram_tensor("selt0", [P, D // P, 2 * P * HYB], BF16,
                             kind="ExternalInput").ap()
    y_dram = nc.dram_tensor("y_all", [NCH * P, D], BF16,
                            kind="ExternalOutput").ap()

    steps = [(e, cp) for e in range(E_LOC) for cp in range(NCP)]
    NS = len(steps)

    with tile.TileContext(nc) as tc:
        with (
            tc.tile_pool(name="const", bufs=1) as const,
            tc.tile_pool(name="work", bufs=2) as spool,
            tc.tile_pool(name="wts", bufs=2) as wpool,
            tc.tile_pool(name="selp", bufs=5) as selpool,
            tc.tile_pool(name="psum", bufs=2, space="PSUM") as ppool,
        ):
            # first-MM critical path: step-0 selT half leads the sync ring
            selt0 = const.tile([P, D // P, 2 * P * HYB], BF16, tag="selt0",
                               name="selt0")
            nc.sync.dma_start(out=selt0[:, :, :2 * P],
                              in_=selt0_d[:, :, :2 * P])
            nc.sync.dma_start(out=selt0[:, :, 2 * P:],
                              in_=selt0_d[:, :, 2 * P:])
            gidx_sb = const.tile([P, GIDX_COLS], I16, tag="gidx",
                                 name="gidx_sb")
            nc.sync.dma_start(out=gidx_sb, in_=gidx)
            b1c_sb = const.tile([P, 2 * E_LOC], F32, tag="b1c",
                                name="b1c_sb")
            nc.scalar.dma_start(out=b1c_sb, in_=b1c_d)
            gates_sb = const.tile([P, NCH], F32, tag="gates",
                                  name="gates_sb")
            nc.scalar.dma_start(out=gates_sb, in_=gates_d)

            wts = {}

            def load_expert(e):
                w1_sb = wpool.tile([P, D // P, H], BF16, tag="w1",
                                   name="w1_sb")
                nc.scalar.dma_start(
                    out=w1_sb,
                    in_=w1t[e].rearrange("(go gi) j -> gi go j", gi=P),
                )
                w2_sb = wpool.tile([P, H // P, D], BF16, tag="w2",
                                   name="w2_sb")
                nc.scalar.dma_start(
                    out=w2_sb,
                    in_=w2t[e].rearrange("(jo ji) o -> ji jo o", ji=P),
                )
                wts[e] = (w1_sb, w2_sb)

            load_expert(0)

            # compute-produced twins of repeatedly-read constants: consumers
            # then wait on precise engine sems instead of coarsened per-ring
            # DMA FIFO sems
            gidx2 = const.tile([P, GIDX_COLS], I16, tag="gidx2", name="gidx2")
            nc.vector.tensor_copy(out=gidx2, in_=gidx_sb)
            b1c2 = const.tile([P, 2 * E_LOC], F32, tag="b1c2", name="b1c2")
            nc.vector.tensor_copy(out=b1c2, in_=b1c_sb)
            gates2 = const.tile([P, NCH], F32, tag="gates2", name="gates2")
            nc.vector.tensor_copy(out=gates2, in_=gates_sb)

            st = {}  # per-step pipeline state

            def stage_a(s):
                e, cp = steps[s]
                if cp == 0 and e + 1 < E_LOC:
                    load_expert(e + 1)
                if s < HYB:
                    # served from the host-pregathered selt0 tile: the
                    # first steps must not wait for the Q7 extended-inst
                    # library that dma_gather needs (~12us install)
                    st[s] = {}
                    return
                # one transposed gather covers this step's 2 chunks:
                # selt[p, g, i] = x[gidx[step*256 + i], g*128 + p]
                selt = selpool.tile([P, D // P, 2 * P], BF16, tag="selt",
                                    name="selt")
                col = (e * NCP + cp) * 16
                nc.gpsimd.dma_gather(
                    selt,
                    x,
                    gidx2[:, col:col + 16],
                    2 * P,
                    2 * P,
                    D,
                    transpose=True,
                )
                st[s] = {"selt": selt}

            def stage_c(s):
                e, cp = steps[s]
                d = st[s]
                if s < HYB:
                    base = 2 * s * P

                    def rslice(g, base=base):
                        return selt0[:, g, base:base + 2 * P]
                else:
                    selt = d["selt"]

                    def rslice(g, selt=selt):
                        return selt[:, g, :]
                hs = []
                for ht in range(2):
                    phT = ppool.tile([P, 2 * P], F32, tag="ph", name="phT",
                                     bufs=3)
                    for g in range(D // P):
                        nc.tensor.matmul(
                            out=phT,
                            lhsT=wts[e][0][:, g, ht * P:(ht + 1) * P],
                            rhs=rslice(g),
                            start=(g == 0),
                            stop=(g == D // P - 1),
                        )
                    h_sb = spool.tile([P, 2 * P], BF16, tag=f"h{ht}",
                                      name="h_sb", bufs=3)
                    nc.scalar.activation(
                        out=h_sb, in_=phT, func=ACT_FUNC,
                        bias=b1c2[:, 2 * e + ht:2 * e + ht + 1],
                    )
                    hs.append(h_sb)
                d["hs"] = hs

            def stage_e(s):
                e, cp = steps[s]
                d = st[s]
                y_out = spool.tile([P, 2, D], BF16, tag="y", name="y_out",
                                   bufs=3)
                last = s == NS - 1
                for hh in range(2):
                    cb = e * NCB + cp * 2 + hh
                    gate_col = gates2[:, cb:cb + 1]
                    for oc in range(D // 512):
                        py = ppool.tile([P, 512], F32, tag="py", name="py",
                                        bufs=5)
                        for ht in range(2):
                            nc.tensor.matmul(
                                out=py,
                                lhsT=d["hs"][ht][:, hh * P:(hh + 1) * P],
                                rhs=wts[e][1][:, ht,
                                              oc * 512:(oc + 1) * 512],
                                start=(ht == 0),
                                stop=(ht == 1),
                            )
                        dst = y_out[:, hh, oc * 512:(oc + 1) * 512]
                        # gate applied during the PSUM->SBUF cast as a
                        # per-partition scale
                        if last and oc >= 2:
                            nc.scalar.activation(out=dst, in_=py,
                                                 func=AF.Copy,
                                                 scale=gate_col)
                        else:
                            nc.vector.tensor_scalar_mul(out=dst, in0=py,
                                                        scalar1=gate_col)
                        if last:
                            rows = slice((2 * s + hh) * P,
                                         (2 * s + hh + 1) * P)
                            nc.sync.dma_start(
                                out=y_dram[rows, oc * 512:(oc + 1) * 512],
                                in_=dst,
                            )
                d["y_out"] = y_out

            def stage_f(s):
                d = st.pop(s)
                if s == NS - 1:
                    return
                for hh in range(2):
                    rows = slice((2 * s + hh) * P, (2 * s + hh + 1) * P)
                    nc.sync.dma_start(out=y_dram[rows],
                                      in_=d["y_out"][:, hh, :])

            # software pipeline with explicit skew offsets (stage, offset)
            sched = [
                (stage_a, 0),
                (stage_c, 2),
                (stage_e, 3),
                (stage_f, 4),
            ]
            max_off = max(off for _, off in sched)
            for si in range(NS + max_off):
                for fn, off in sched:
                    s = si - off
                    if 0 <= s < NS:
                        fn(s)
    nc.compile()
    return nc


_CACHE = {}
_CACHE_LOCK = threading.Lock()


def _get_nc():
    with _CACHE_LOCK:
        if "nc" not in _CACHE:
            _CACHE["nc"] = build_kernel()
        return _CACHE["nc"]


def _prep_indices(idx_e, gate_e):
    """Dedup one expert's slots: unique tokens (ascending) + summed gates.

    Returns (gid (C,) int16 gather idx, uniq (nu,) token ids,
    gate (C,) f32).  Pad slots gather token 0 with gate 0 (their y rows
    are exact zeros and the host ignores them).
    """
    uniq, inv = np.unique(idx_e, return_inverse=True)
    gsum = np.zeros(len(uniq), dtype=np.float64)
    np.add.at(gsum, inv, gate_e.astype(np.float64))
    nu = len(uniq)
    assert nu <= C
    gid = np.zeros(C, dtype=np.int16)
    gid[:nu] = uniq.astype(np.int16)
    gate = np.zeros(C, dtype=np.float32)
    gate[:nu] = gsum.astype(np.float32)
    return gid, uniq, gate


def _make_in_maps(x, W1, b1, W2, b2, expert_indices, expert_gate):
    bf16 = ml_dtypes.bfloat16
    x = np.asarray(x, dtype=np.float32)
    W1 = np.asarray(W1, dtype=np.float32)
    b1f = np.asarray(b1, dtype=np.float32)
    W2 = np.asarray(W2, dtype=np.float32)
    idx_all = np.asarray(expert_indices, dtype=np.int32)
    gate_all = np.asarray(expert_gate, dtype=np.float32)

    x_bf = [np.ascontiguousarray(x[bb].astype(bf16)) for bb in range(B)]
    w1t_bf = np.ascontiguousarray(W1.transpose(0, 2, 1).astype(bf16))
    w2t_bf = np.ascontiguousarray(W2.transpose(0, 2, 1).astype(bf16))

    in_maps = []
    uniqs = []  # per core: list of E_LOC unique-token arrays (host combine)
    for core in range(N_CORES):
        bb, half = divmod(core, 2)
        es = slice(half * E_LOC, half * E_LOC + E_LOC)
        gidx_t = np.empty((P, GIDX_COLS), dtype=np.int16)
        selt0_t = None
        gates_t = np.empty((P, NCH), dtype=np.float32)
        core_uniqs = []
        for le, ge in enumerate(range(half * E_LOC, half * E_LOC + E_LOC)):
            gid, uniq, gate = _prep_indices(idx_all[bb, ge], gate_all[bb, ge])
            core_uniqs.append(uniq)
            # wrapped int16 gather layout: slot i -> [i % 16, base + i // 16]
            gidx_t[:, le * (C // 16):(le + 1) * (C // 16)] = \
                np.tile(gid.reshape(C // 16, 16).T, (P // 16, 1))
            if le == 0:
                xs = x_bf[bb][gid[:2 * HYB * P].astype(np.int64)]
                selt0_t = np.ascontiguousarray(
                    xs.reshape(2 * HYB * P, D // P, P).transpose(2, 1, 0))
            gates_t[:, le * NCB:(le + 1) * NCB] = gate.reshape(NCB, P).T
        # per-partition bias columns: b1c[p, 2e+ht] = b1[e, ht*128+p]
        b1c_t = np.ascontiguousarray(
            b1f[es].reshape(E_LOC * 2, P).T.astype(np.float32))
        uniqs.append(core_uniqs)
        in_maps.append(
            {
                "x": x_bf[bb],
                "w1t": np.ascontiguousarray(w1t_bf[es]),
                "w2t": np.ascontiguousarray(w2t_bf[es]),
                "b1c": b1c_t,
                "gates": np.ascontiguousarray(gates_t),
                "gidx": np.ascontiguousarray(gidx_t),
                "selt0": selt0_t,
            }
        )
    return in_maps, uniqs


def kernel(x, W1, b1, W2, b2, expert_indices, expert_gate, num_tokens, *,
           _trace=False, _trace_kwargs=None):
    assert int(num_tokens) == T
    nc = _get_nc()
    in_maps, uniqs = _make_in_maps(x, W1, b1, W2, b2, expert_indices,
                                   expert_gate)
    res = run_bass_kernel_spmd(
        nc,
        in_maps,
        core_ids=list(range(N_CORES)),
        trace=_trace,
        **(_trace_kwargs or {}),
    )
    idx_all = np.asarray(expert_indices)
    gate_all = np.asarray(expert_gate, dtype=np.float64)
    b2f = np.asarray(b2, dtype=np.float32)
    full = np.empty((B, T, D), dtype=np.float32)
    for bb in range(B):
        acc = np.zeros((T, D), dtype=np.float32)
        for half in range(2):
            core = 2 * bb + half
            y_all = np.asarray(res.results[core]["y_all"],
                               dtype=np.float32).reshape(NCH, P, D)
            for e in range(E_LOC):
                uniq = uniqs[core][e]
                y_e = y_all[e * NCB:(e + 1) * NCB].reshape(C, D)
                acc[uniq] += y_e[:len(uniq)]
        # shared output bias, weighted by each token's total gate mass
        total_gate = np.bincount(
            idx_all[bb].ravel().astype(np.int64),
            weights=gate_all[bb].ravel(),
            minlength=T,
        ).astype(np.float32)
        acc += total_gate[:, None] * b2f[None, :]
        full[bb] = acc
    if _trace:
        kernel.last_results = res
    return full


# revision 16
# speedup vs baseline: 1.1331x; 1.0647x over previous
"""Experts-choose MoE MLP kernel for 8 TRN2 NeuronCores — bf16 pipeline v10.

Sharding: core = 2*b + half handles batch row b and experts
[4*half, 4*half+4). Each core writes its 4096 gated expert-output rows
CONTIGUOUSLY to DRAM (y_all); the host places them into the (T, D)
output during unsharding (strictly less host arithmetic than summing
dense partial buffers) and adds the shared-bias term
total_gate (x) b2, which never touches the device.

Host prep: per (core, expert) the C=1024 slots are deduplicated to
unique token ids (ascending) with summed gates.  Pad slots (beyond the
unique count) gather token 0 with gate 0 — their y rows are exact
zeros and the host ignores them.  No scatter, no merge matrices.

Per 256-token step on device (4-stage software pipeline):
  A: dma_gather(transpose=True) — gathers x rows from HBM and writes
     them TRANSPOSED (selT layout [d128, d16, c256]) in one SWDGE op.
     The first HYB steps instead read a host-pregathered selt0 tile so
     the PE starts ~13us in while the Q7 extended-instruction library
     (~12us install, needed by dma_gather) loads in their shadow.
  C: GEMM1 transposed: phT[h128, c256] = sum_g w1[g,ht]^T @ selT[g]
     (weights stationary); Gelu with b1 fused as the ScalarE
     activation's PER-PARTITION bias -> h_T bf16 [h, c].  No separate
     bias matmul, no h transpose needed later.
  E: GEMM2 straight from h_T slices (lhsT = h_T[:, chunk]); the gate
     is applied during the PSUM->SBUF cast as a per-partition
     tensor_scalar_mul on DVE.  No merge matmul, no extra cast.
  F: contiguous HWDGE store of the step's 256 y rows (per chunk); the
     last step stores each 512-wide slice as it is cast for a fast
     drain.
Weight loads go on the scalar HWDGE ring (w1/w2 of expert e+1 prefetch
during expert e); index/selt0 loads go first on the sync ring.
"""

import threading

import numpy as np
import ml_dtypes

import concourse.mybir as mybir
import concourse.tile as tile
from concourse import bacc
from concourse.bass_utils import run_bass_kernel_spmd

P = 128
B, T, D, E, C = 4, 4096, 2048, 8, 1024
H = 256
E_LOC = 4
NCB = C // P          # 8 chunks per expert
NCP = NCB // 2        # 4 steps per expert (2 chunks per step)
NCH = E_LOC * NCB     # 32 chunks per core
N_CORES = 8

BF16 = mybir.dt.bfloat16
F32 = mybir.dt.float32
I16 = mybir.dt.int16
AF = mybir.ActivationFunctionType

GIDX_COLS = E_LOC * C // 16   # 256 int16 columns (wrapped in 16 partitions)
HYB = 2                       # startup steps served by host-pregathered selT

ACT_FUNC = AF.Gelu  # sim_debug overrides (interp lacks Gelu)


def build_kernel():
    nc = bacc.Bacc("TRN2", target_bir_lowering=False, debug=False)

    x = nc.dram_tensor("x", [T, D], BF16, kind="ExternalInput").ap()
    w1t = nc.dram_tensor("w1t", [E_LOC, D, H], BF16, kind="ExternalInput").ap()
    w2t = nc.dram_tensor("w2t", [E_LOC, H, D], BF16, kind="ExternalInput").ap()
    b1c_d = nc.dram_tensor("b1c", [P, 2 * E_LOC], F32,
                           kind="ExternalInput").ap()
    gates_d = nc.dram_tensor("gates", [P, NCH], F32,
                             kind="ExternalInput").ap()
    gidx = nc.dram_tensor("gidx", [P, GIDX_COLS], I16,
                          kind="ExternalInput").ap()
    selt0_d = nc.dram_tensor("selt0", [P, HYB, D // P, 2 * P], BF16,
                             kind="ExternalInput").ap()
    y_dram = nc.dram_tensor("y_all", [NCH * P, D], BF16,
                            kind="ExternalOutput").ap()

    steps = [(e, cp) for e in range(E_LOC) for cp in range(NCP)]
    NS = len(steps)

    with tile.TileContext(nc) as tc:
        with (
            tc.tile_pool(name="const", bufs=1) as const,
            tc.tile_pool(name="work", bufs=2) as spool,
            tc.tile_pool(name="wts", bufs=2) as wpool,
            tc.tile_pool(name="selp", bufs=5) as selpool,
            tc.tile_pool(name="psum", bufs=2, space="PSUM") as ppool,
        ):
            # first-MM critical path: step-0 selT half leads the sync ring
            selt0 = const.tile([P, HYB, D // P, 2 * P], BF16, tag="selt0",
                               name="selt0")
            nc.sync.dma_start(out=selt0[:, 0], in_=selt0_d[:, 0])
            gidx_sb = const.tile([P, GIDX_COLS], I16, tag="gidx",
                                 name="gidx_sb")
            nc.sync.dma_start(out=gidx_sb, in_=gidx)
            nc.sync.dma_start(out=selt0[:, 1], in_=selt0_d[:, 1])
            b1c_sb = const.tile([P, 2 * E_LOC], F32, tag="b1c",
                                name="b1c_sb")
            nc.scalar.dma_start(out=b1c_sb, in_=b1c_d)
            gates_sb = const.tile([P, NCH], F32, tag="gates",
                                  name="gates_sb")
            nc.scalar.dma_start(out=gates_sb, in_=gates_d)

            wts = {}

            def load_expert(e):
                w1_sb = wpool.tile([P, D // P, H], BF16, tag="w1",
                                   name="w1_sb")
                nc.scalar.dma_start(
                    out=w1_sb,
                    in_=w1t[e].rearrange("(go gi) j -> gi go j", gi=P),
                )
                w2_sb = wpool.tile([P, H // P, D], BF16, tag="w2",
                                   name="w2_sb")
                nc.scalar.dma_start(
                    out=w2_sb,
                    in_=w2t[e].rearrange("(jo ji) o -> ji jo o", ji=P),
                )
                wts[e] = (w1_sb, w2_sb)

            load_expert(0)

            # compute-produced twins of repeatedly-read constants: consumers
            # then wait on precise engine sems instead of coarsened per-ring
            # DMA FIFO sems
            gidx2 = const.tile([P, GIDX_COLS], I16, tag="gidx2", name="gidx2")
            nc.vector.tensor_copy(out=gidx2, in_=gidx_sb)
            b1c2 = const.tile([P, 2 * E_LOC], F32, tag="b1c2", name="b1c2")
            nc.vector.tensor_copy(out=b1c2, in_=b1c_sb)
            gates2 = const.tile([P, NCH], F32, tag="gates2", name="gates2")
            nc.vector.tensor_copy(out=gates2, in_=gates_sb)

            st = {}  # per-step pipeline state

            def stage_a(s):
                e, cp = steps[s]
                if cp == 0 and e + 1 < E_LOC:
                    load_expert(e + 1)
                if s < HYB:
                    # served from the host-pregathered selt0 tile: the
                    # first steps must not wait for the Q7 extended-inst
                    # library that dma_gather needs (~12us install)
                    st[s] = {}
                    return
                # one transposed gather covers this step's 2 chunks:
                # selt[p, g, i] = x[gidx[step*256 + i], g*128 + p]
                selt = selpool.tile([P, D // P, 2 * P], BF16, tag="selt",
                                    name="selt")
                col = (e * NCP + cp) * 16
                nc.gpsimd.dma_gather(
                    selt,
                    x,
                    gidx2[:, col:col + 16],
                    2 * P,
                    2 * P,
                    D,
                    transpose=True,
                )
                st[s] = {"selt": selt}

            def stage_c(s):
                e, cp = steps[s]
                d = st[s]
                if s < HYB:

                    def rslice(g, s=s):
                        return selt0[:, s, g, :]
                else:
                    selt = d["selt"]

                    def rslice(g, selt=selt):
                        return selt[:, g, :]
                hs = []
                for ht in range(2):
                    phT = ppool.tile([P, 2 * P], F32, tag="ph", name="phT",
                                     bufs=3)
                    for g in range(D // P):
                        nc.tensor.matmul(
                            out=phT,
                            lhsT=wts[e][0][:, g, ht * P:(ht + 1) * P],
                            rhs=rslice(g),
                            start=(g == 0),
                            stop=(g == D // P - 1),
                        )
                    h_sb = spool.tile([P, 2 * P], BF16, tag=f"h{ht}",
                                      name="h_sb", bufs=3)
                    nc.scalar.activation(
                        out=h_sb, in_=phT, func=ACT_FUNC,
                        bias=b1c2[:, 2 * e + ht:2 * e + ht + 1],
                    )
                    hs.append(h_sb)
                d["hs"] = hs

            def stage_e(s):
                e, cp = steps[s]
                d = st[s]
                y_out = spool.tile([P, 2, D], BF16, tag="y", name="y_out",
                                   bufs=3)
                last = s == NS - 1
                for hh in range(2):
                    cb = e * NCB + cp * 2 + hh
                    gate_col = gates2[:, cb:cb + 1]
                    for oc in range(D // 512):
                        py = ppool.tile([P, 512], F32, tag="py", name="py",
                                        bufs=5)
                        for ht in range(2):
                            nc.tensor.matmul(
                                out=py,
                                lhsT=d["hs"][ht][:, hh * P:(hh + 1) * P],
                                rhs=wts[e][1][:, ht,
                                              oc * 512:(oc + 1) * 512],
                                start=(ht == 0),
                                stop=(ht == 1),
                            )
                        dst = y_out[:, hh, oc * 512:(oc + 1) * 512]
                        # gate applied during the PSUM->SBUF cast as a
                        # per-partition scale
                        if oc == 3 or (last and oc >= 2):
                            nc.scalar.activation(out=dst, in_=py,
                                                 func=AF.Copy,
                                                 scale=gate_col)
                        else:
                            nc.vector.tensor_scalar_mul(out=dst, in0=py,
                                                        scalar1=gate_col)
                        if last:
                            rows = slice((2 * s + hh) * P,
                                         (2 * s + hh + 1) * P)
                            nc.sync.dma_start(
                                out=y_dram[rows, oc * 512:(oc + 1) * 512],
                                in_=dst,
                            )
                d["y_out"] = y_out

            def stage_f(s):
                d = st.pop(s)
                if s == NS - 1:
                    return
                for hh in range(2):
                    rows = slice((2 * s + hh) * P, (2 * s + hh + 1) * P)
                    nc.sync.dma_start(out=y_dram[rows],
                                      in_=d["y_out"][:, hh, :])

            # software pipeline with explicit skew offsets (stage, offset)
            sched = [
                (stage_a, 0),
                (stage_c, 2),
                (stage_e, 3),
                (stage_f, 4),
            ]
            max_off = max(off for _, off in sched)
            for si in range(NS + max_off):
                for fn, off in sched:
                    s = si - off
                    if 0 <= s < NS:
                        fn(s)
    nc.compile()
    return nc


_CACHE = {}
_CACHE_LOCK = threading.Lock()


def _get_nc():
    with _CACHE_LOCK:
        if "nc" not in _CACHE:
            _CACHE["nc"] = build_kernel()
        return _CACHE["nc"]


def _prep_indices(idx_e, gate_e):
    """Dedup one expert's slots: unique tokens (ascending) + summed gates.

    Returns (gid (C,) int16 gather idx, uniq (nu,) token ids,
    gate (C,) f32).  Pad slots gather token 0 with gate 0 (their y rows
    are exact zeros and the host ignores them).
    """
    uniq, inv = np.unique(idx_e, return_inverse=True)
    gsum = np.zeros(len(uniq), dtype=np.float64)
    np.add.at(gsum, inv, gate_e.astype(np.float64))
    nu = len(uniq)
    assert nu <= C
    gid = np.zeros(C, dtype=np.int16)
    gid[:nu] = uniq.astype(np.int16)
    gate = np.zeros(C, dtype=np.float32)
    gate[:nu] = gsum.astype(np.float32)
    return gid, uniq, gate


def _make_in_maps(x, W1, b1, W2, b2, expert_indices, expert_gate):
    bf16 = ml_dtypes.bfloat16
    x = np.asarray(x, dtype=np.float32)
    W1 = np.asarray(W1, dtype=np.float32)
    b1f = np.asarray(b1, dtype=np.float32)
    W2 = np.asarray(W2, dtype=np.float32)
    idx_all = np.asarray(expert_indices, dtype=np.int32)
    gate_all = np.asarray(expert_gate, dtype=np.float32)

    x_bf = [np.ascontiguousarray(x[bb].astype(bf16)) for bb in range(B)]
    w1t_bf = np.ascontiguousarray(W1.transpose(0, 2, 1).astype(bf16))
    w2t_bf = np.ascontiguousarray(W2.transpose(0, 2, 1).astype(bf16))

    in_maps = []
    uniqs = []  # per core: list of E_LOC unique-token arrays (host combine)
    for core in range(N_CORES):
        bb, half = divmod(core, 2)
        es = slice(half * E_LOC, half * E_LOC + E_LOC)
        gidx_t = np.empty((P, GIDX_COLS), dtype=np.int16)
        selt0_t = None
        gates_t = np.empty((P, NCH), dtype=np.float32)
        core_uniqs = []
        for le, ge in enumerate(range(half * E_LOC, half * E_LOC + E_LOC)):
            gid, uniq, gate = _prep_indices(idx_all[bb, ge], gate_all[bb, ge])
            core_uniqs.append(uniq)
            # wrapped int16 gather layout: slot i -> [i % 16, base + i // 16]
            gidx_t[:, le * (C // 16):(le + 1) * (C // 16)] = \
                np.tile(gid.reshape(C // 16, 16).T, (P // 16, 1))
            if le == 0:
                xs = x_bf[bb][gid[:2 * HYB * P].astype(np.int64)]
                # [p, s, g, c] with c the in-step slot index
                selt0_t = np.ascontiguousarray(
                    xs.reshape(HYB, 2 * P, D // P, P).transpose(3, 0, 2, 1))
            gates_t[:, le * NCB:(le + 1) * NCB] = gate.reshape(NCB, P).T
        # per-partition bias columns: b1c[p, 2e+ht] = b1[e, ht*128+p]
        b1c_t = np.ascontiguousarray(
            b1f[es].reshape(E_LOC * 2, P).T.astype(np.float32))
        uniqs.append(core_uniqs)
        in_maps.append(
            {
                "x": x_bf[bb],
                "w1t": np.ascontiguousarray(w1t_bf[es]),
                "w2t": np.ascontiguousarray(w2t_bf[es]),
                "b1c": b1c_t,
                "gates": np.ascontiguousarray(gates_t),
                "gidx": np.ascontiguousarray(gidx_t),
                "selt0": selt0_t,
            }
        )
    return in_maps, uniqs


def kernel(x, W1, b1, W2, b2, expert_indices, expert_gate, num_tokens, *,
           _trace=False, _trace_kwargs=None):
    assert int(num_tokens) == T
    nc = _get_nc()
    in_maps, uniqs = _make_in_maps(x, W1, b1, W2, b2, expert_indices,
                                   expert_gate)
    res = run_bass_kernel_spmd(
        nc,
        in_maps,
        core_ids=list(range(N_CORES)),
        trace=_trace,
        **(_trace_kwargs or {}),
    )
    idx_all = np.asarray(expert_indices)
    gate_all = np.asarray(expert_gate, dtype=np.float64)
    b2f = np.asarray(b2, dtype=np.float32)
    full = np.empty((B, T, D), dtype=np.float32)
    for bb in range(B):
        acc = np.zeros((T, D), dtype=np.float32)
        for half in range(2):
            core = 2 * bb + half
            y_all = np.asarray(res.results[core]["y_all"],
                               dtype=np.float32).reshape(NCH, P, D)
            for e in range(E_LOC):
                uniq = uniqs[core][e]
                y_e = y_all[e * NCB:(e + 1) * NCB].reshape(C, D)
                acc[uniq] += y_e[:len(uniq)]
        # shared output bias, weighted by each token's total gate mass
        total_gate = np.bincount(
            idx_all[bb].ravel().astype(np.int64),
            weights=gate_all[bb].ravel(),
            minlength=T,
        ).astype(np.float32)
        acc += total_gate[:, None] * b2f[None, :]
        full[bb] = acc
    if _trace:
        kernel.last_results = res
    return full


# revision 17
# speedup vs baseline: 1.1349x; 1.0016x over previous
"""Experts-choose MoE MLP kernel for 8 TRN2 NeuronCores — bf16 pipeline v10.

Sharding: core = 2*b + half handles batch row b and experts
[4*half, 4*half+4). Each core writes its 4096 gated expert-output rows
CONTIGUOUSLY to DRAM (y_all); the host places them into the (T, D)
output during unsharding (strictly less host arithmetic than summing
dense partial buffers) and adds the shared-bias term
total_gate (x) b2, which never touches the device.

Host prep: per (core, expert) the C=1024 slots are deduplicated to
unique token ids (ascending) with summed gates.  Pad slots (beyond the
unique count) gather token 0 with gate 0 — their y rows are exact
zeros and the host ignores them.  No scatter, no merge matrices.

Per 256-token step on device (4-stage software pipeline):
  A: dma_gather(transpose=True) — gathers x rows from HBM and writes
     them TRANSPOSED (selT layout [d128, d16, c256]) in one SWDGE op.
     The first HYB steps instead read a host-pregathered selt0 tile so
     the PE starts ~13us in while the Q7 extended-instruction library
     (~12us install, needed by dma_gather) loads in their shadow.
  C: GEMM1 transposed: phT[h128, c256] = sum_g w1[g,ht]^T @ selT[g]
     (weights stationary); Gelu with b1 fused as the ScalarE
     activation's PER-PARTITION bias -> h_T bf16 [h, c].  No separate
     bias matmul, no h transpose needed later.
  E: GEMM2 straight from h_T slices (lhsT = h_T[:, chunk]); the gate
     is applied during the PSUM->SBUF cast as a per-partition
     tensor_scalar_mul on DVE.  No merge matmul, no extra cast.
  F: contiguous HWDGE store of the step's 256 y rows (per chunk); the
     last step stores each 512-wide slice as it is cast for a fast
     drain.
Weight loads go on the scalar HWDGE ring (w1/w2 of expert e+1 prefetch
during expert e); index/selt0 loads go first on the sync ring.
"""

import threading

import numpy as np
import ml_dtypes

import concourse.mybir as mybir
import concourse.tile as tile
from concourse import bacc
from concourse.bass_utils import run_bass_kernel_spmd

P = 128
B, T, D, E, C = 4, 4096, 2048, 8, 1024
H = 256
E_LOC = 4
NCB = C // P          # 8 chunks per expert
NCP = NCB // 2        # 4 steps per expert (2 chunks per step)
NCH = E_LOC * NCB     # 32 chunks per core
N_CORES = 8

BF16 = mybir.dt.bfloat16
F32 = mybir.dt.float32
I16 = mybir.dt.int16
AF = mybir.ActivationFunctionType

GIDX_COLS = E_LOC * C // 16   # 256 int16 columns (wrapped in 16 partitions)
HYB = 2                       # startup steps served by host-pregathered selT

ACT_FUNC = AF.Gelu  # sim_debug overrides (interp lacks Gelu)


def build_kernel():
    nc = bacc.Bacc("TRN2", target_bir_lowering=False, debug=False)

    x = nc.dram_tensor("x", [T, D], BF16, kind="ExternalInput").ap()
    w1t = nc.dram_tensor("w1t", [E_LOC, D, H], BF16, kind="ExternalInput").ap()
    w2t = nc.dram_tensor("w2t", [E_LOC, H, D], BF16, kind="ExternalInput").ap()
    b1c_d = nc.dram_tensor("b1c", [P, 2 * E_LOC], F32,
                           kind="ExternalInput").ap()
    gates_d = nc.dram_tensor("gates", [P, NCH], F32,
                             kind="ExternalInput").ap()
    gidx = nc.dram_tensor("gidx", [P, GIDX_COLS], I16,
                          kind="ExternalInput").ap()
    selt0_d = nc.dram_tensor("selt0", [P, HYB, D // P, 2 * P], BF16,
                             kind="ExternalInput").ap()
    y_dram = nc.dram_tensor("y_all", [NCH * P, D], BF16,
                            kind="ExternalOutput").ap()

    steps = [(e, cp) for e in range(E_LOC) for cp in range(NCP)]
    NS = len(steps)

    with tile.TileContext(nc) as tc:
        with (
            tc.tile_pool(name="const", bufs=1) as const,
            tc.tile_pool(name="work", bufs=2) as spool,
            tc.tile_pool(name="wts", bufs=2) as wpool,
            tc.tile_pool(name="selp", bufs=5) as selpool,
            tc.tile_pool(name="psum", bufs=2, space="PSUM") as ppool,
        ):
            # first-MM critical path, serialized in need order on the
            # sync ring (rings do not share HBM bandwidth fairly): w1,
            # then the step-0 selT half, then step-1's
            selt0 = const.tile([P, HYB, D // P, 2 * P], BF16, tag="selt0",
                               name="selt0")
            gidx_sb = const.tile([P, GIDX_COLS], I16, tag="gidx",
                                 name="gidx_sb")

            def load_expert(e, ring=None):
                ring = ring or nc.scalar
                w1_sb = wpool.tile([P, D // P, H], BF16, tag="w1",
                                   name="w1_sb")
                ring.dma_start(
                    out=w1_sb,
                    in_=w1t[e].rearrange("(go gi) j -> gi go j", gi=P),
                )
                w2_sb = wpool.tile([P, H // P, D], BF16, tag="w2",
                                   name="w2_sb")
                ring.dma_start(
                    out=w2_sb,
                    in_=w2t[e].rearrange("(jo ji) o -> ji jo o", ji=P),
                )
                wts[e] = (w1_sb, w2_sb)

            wts = {}
            # sync ring: w1(e0) -> selt0 halves -> gidx
            w1_sb0 = wpool.tile([P, D // P, H], BF16, tag="w1",
                                name="w1_sb")
            nc.sync.dma_start(
                out=w1_sb0,
                in_=w1t[0].rearrange("(go gi) j -> gi go j", gi=P),
            )
            nc.sync.dma_start(out=selt0[:, 0], in_=selt0_d[:, 0])
            nc.sync.dma_start(out=selt0[:, 1], in_=selt0_d[:, 1])
            nc.sync.dma_start(out=gidx_sb, in_=gidx)
            # scalar ring: w2(e0) + small constants
            b1c_sb = const.tile([P, 2 * E_LOC], F32, tag="b1c",
                                name="b1c_sb")
            nc.scalar.dma_start(out=b1c_sb, in_=b1c_d)
            w2_sb0 = wpool.tile([P, H // P, D], BF16, tag="w2",
                                name="w2_sb")
            nc.scalar.dma_start(
                out=w2_sb0,
                in_=w2t[0].rearrange("(jo ji) o -> ji jo o", ji=P),
            )
            gates_sb = const.tile([P, NCH], F32, tag="gates",
                                  name="gates_sb")
            nc.scalar.dma_start(out=gates_sb, in_=gates_d)
            wts[0] = (w1_sb0, w2_sb0)

            # compute-produced twins of repeatedly-read constants: consumers
            # then wait on precise engine sems instead of coarsened per-ring
            # DMA FIFO sems
            gidx2 = const.tile([P, GIDX_COLS], I16, tag="gidx2", name="gidx2")
            nc.vector.tensor_copy(out=gidx2, in_=gidx_sb)
            b1c2 = const.tile([P, 2 * E_LOC], F32, tag="b1c2", name="b1c2")
            nc.vector.tensor_copy(out=b1c2, in_=b1c_sb)
            gates2 = const.tile([P, NCH], F32, tag="gates2", name="gates2")
            nc.vector.tensor_copy(out=gates2, in_=gates_sb)

            st = {}  # per-step pipeline state

            def stage_a(s):
                e, cp = steps[s]
                if cp == 0 and e + 1 < E_LOC:
                    load_expert(e + 1)
                if s < HYB:
                    # served from the host-pregathered selt0 tile: the
                    # first steps must not wait for the Q7 extended-inst
                    # library that dma_gather needs (~12us install)
                    st[s] = {}
                    return
                # one transposed gather covers this step's 2 chunks:
                # selt[p, g, i] = x[gidx[step*256 + i], g*128 + p]
                selt = selpool.tile([P, D // P, 2 * P], BF16, tag="selt",
                                    name="selt")
                col = (e * NCP + cp) * 16
                nc.gpsimd.dma_gather(
                    selt,
                    x,
                    gidx2[:, col:col + 16],
                    2 * P,
                    2 * P,
                    D,
                    transpose=True,
                )
                st[s] = {"selt": selt}

            def stage_c(s):
                e, cp = steps[s]
                d = st[s]
                if s < HYB:

                    def rslice(g, s=s):
                        return selt0[:, s, g, :]
                else:
                    selt = d["selt"]

                    def rslice(g, selt=selt):
                        return selt[:, g, :]
                hs = []
                for ht in range(2):
                    phT = ppool.tile([P, 2 * P], F32, tag="ph", name="phT",
                                     bufs=2)
                    for g in range(D // P):
                        nc.tensor.matmul(
                            out=phT,
                            lhsT=wts[e][0][:, g, ht * P:(ht + 1) * P],
                            rhs=rslice(g),
                            start=(g == 0),
                            stop=(g == D // P - 1),
                        )
                    h_sb = spool.tile([P, 2 * P], BF16, tag=f"h{ht}",
                                      name="h_sb", bufs=3)
                    nc.scalar.activation(
                        out=h_sb, in_=phT, func=ACT_FUNC,
                        bias=b1c2[:, 2 * e + ht:2 * e + ht + 1],
                    )
                    hs.append(h_sb)
                d["hs"] = hs

            def stage_e(s):
                e, cp = steps[s]
                d = st[s]
                y_out = spool.tile([P, 2, D], BF16, tag="y", name="y_out",
                                   bufs=3)
                last = s == NS - 1
                for hh in range(2):
                    cb = e * NCB + cp * 2 + hh
                    gate_col = gates2[:, cb:cb + 1]
                    for oc in range(D // 512):
                        py = ppool.tile([P, 512], F32, tag="py", name="py",
                                        bufs=6)
                        for ht in range(2):
                            nc.tensor.matmul(
                                out=py,
                                lhsT=d["hs"][ht][:, hh * P:(hh + 1) * P],
                                rhs=wts[e][1][:, ht,
                                              oc * 512:(oc + 1) * 512],
                                start=(ht == 0),
                                stop=(ht == 1),
                            )
                        dst = y_out[:, hh, oc * 512:(oc + 1) * 512]
                        # gate applied during the PSUM->SBUF cast as a
                        # per-partition scale
                        if oc == 3 or (last and oc >= 2):
                            nc.scalar.activation(out=dst, in_=py,
                                                 func=AF.Copy,
                                                 scale=gate_col)
                        else:
                            nc.vector.tensor_scalar_mul(out=dst, in0=py,
                                                        scalar1=gate_col)
                        if last:
                            rows = slice((2 * s + hh) * P,
                                         (2 * s + hh + 1) * P)
                            nc.sync.dma_start(
                                out=y_dram[rows, oc * 512:(oc + 1) * 512],
                                in_=dst,
                            )
                d["y_out"] = y_out

            def stage_f(s):
                d = st.pop(s)
                if s == NS - 1:
                    return
                for hh in range(2):
                    rows = slice((2 * s + hh) * P, (2 * s + hh + 1) * P)
                    nc.sync.dma_start(out=y_dram[rows],
                                      in_=d["y_out"][:, hh, :])

            # software pipeline with explicit skew offsets (stage, offset)
            sched = [
                (stage_a, 0),
                (stage_c, 2),
                (stage_e, 3),
                (stage_f, 4),
            ]
            max_off = max(off for _, off in sched)
            for si in range(NS + max_off):
                for fn, off in sched:
                    s = si - off
                    if 0 <= s < NS:
                        fn(s)
    nc.compile()
    return nc


_CACHE = {}
_CACHE_LOCK = threading.Lock()


def _get_nc():
    with _CACHE_LOCK:
        if "nc" not in _CACHE:
            _CACHE["nc"] = build_kernel()
        return _CACHE["nc"]


def _prep_indices(idx_e, gate_e):
    """Dedup one expert's slots: unique tokens (ascending) + summed gates.

    Returns (gid (C,) int16 gather idx, uniq (nu,) token ids,
    gate (C,) f32).  Pad slots gather token 0 with gate 0 (their y rows
    are exact zeros and the host ignores them).
    """
    uniq, inv = np.unique(idx_e, return_inverse=True)
    gsum = np.zeros(len(uniq), dtype=np.float64)
    np.add.at(gsum, inv, gate_e.astype(np.float64))
    nu = len(uniq)
    assert nu <= C
    gid = np.zeros(C, dtype=np.int16)
    gid[:nu] = uniq.astype(np.int16)
    gate = np.zeros(C, dtype=np.float32)
    gate[:nu] = gsum.astype(np.float32)
    return gid, uniq, gate


def _make_in_maps(x, W1, b1, W2, b2, expert_indices, expert_gate):
    bf16 = ml_dtypes.bfloat16
    x = np.asarray(x, dtype=np.float32)
    W1 = np.asarray(W1, dtype=np.float32)
    b1f = np.asarray(b1, dtype=np.float32)
    W2 = np.asarray(W2, dtype=np.float32)
    idx_all = np.asarray(expert_indices, dtype=np.int32)
    gate_all = np.asarray(expert_gate, dtype=np.float32)

    x_bf = [np.ascontiguousarray(x[bb].astype(bf16)) for bb in range(B)]
    w1t_bf = np.ascontiguousarray(W1.transpose(0, 2, 1).astype(bf16))
    w2t_bf = np.ascontiguousarray(W2.transpose(0, 2, 1).astype(bf16))

    in_maps = []
    uniqs = []  # per core: list of E_LOC unique-token arrays (host combine)
    for core in range(N_CORES):
        bb, half = divmod(core, 2)
        es = slice(half * E_LOC, half * E_LOC + E_LOC)
        gidx_t = np.empty((P, GIDX_COLS), dtype=np.int16)
        selt0_t = None
        gates_t = np.empty((P, NCH), dtype=np.float32)
        core_uniqs = []
        for le, ge in enumerate(range(half * E_LOC, half * E_LOC + E_LOC)):
            gid, uniq, gate = _prep_indices(idx_all[bb, ge], gate_all[bb, ge])
            core_uniqs.append(uniq)
            # wrapped int16 gather layout: slot i -> [i % 16, base + i // 16]
            gidx_t[:, le * (C // 16):(le + 1) * (C // 16)] = \
                np.tile(gid.reshape(C // 16, 16).T, (P // 16, 1))
            if le == 0:
                xs = x_bf[bb][gid[:2 * HYB * P].astype(np.int64)]
                # [p, s, g, c] with c the in-step slot index
                selt0_t = np.ascontiguousarray(
                    xs.reshape(HYB, 2 * P, D // P, P).transpose(3, 0, 2, 1))
            gates_t[:, le * NCB:(le + 1) * NCB] = gate.reshape(NCB, P).T
        # per-partition bias columns: b1c[p, 2e+ht] = b1[e, ht*128+p]
        b1c_t = np.ascontiguousarray(
            b1f[es].reshape(E_LOC * 2, P).T.astype(np.float32))
        uniqs.append(core_uniqs)
        in_maps.append(
            {
                "x": x_bf[bb],
                "w1t": np.ascontiguousarray(w1t_bf[es]),
                "w2t": np.ascontiguousarray(w2t_bf[es]),
                "b1c": b1c_t,
                "gates": np.ascontiguousarray(gates_t),
                "gidx": np.ascontiguousarray(gidx_t),
                "selt0": selt0_t,
            }
        )
    return in_maps, uniqs


def kernel(x, W1, b1, W2, b2, expert_indices, expert_gate, num_tokens, *,
           _trace=False, _trace_kwargs=None):
    assert int(num_tokens) == T
    nc = _get_nc()
    in_maps, uniqs = _make_in_maps(x, W1, b1, W2, b2, expert_indices,
                                   expert_gate)
    res = run_bass_kernel_spmd(
        nc,
        in_maps,
        core_ids=list(range(N_CORES)),
        trace=_trace,
        **(_trace_kwargs or {}),
    )
    idx_all = np.asarray(expert_indices)
    gate_all = np.asarray(expert_gate, dtype=np.float64)
    b2f = np.asarray(b2, dtype=np.float32)
    full = np.empty((B, T, D), dtype=np.float32)
    for bb in range(B):
        acc = np.zeros((T, D), dtype=np.float32)
        for half in range(2):
            core = 2 * bb + half
            y_all = np.asarray(res.results[core]["y_all"],
                               dtype=np.float32).reshape(NCH, P, D)
            for e in range(E_LOC):
                uniq = uniqs[core][e]
                y_e = y_all[e * NCB:(e + 1) * NCB].reshape(C, D)
                acc[uniq] += y_e[:len(uniq)]
        # shared output bias, weighted by each token's total gate mass
        total_gate = np.bincount(
            idx_all[bb].ravel().astype(np.int64),
            weights=gate_all[bb].ravel(),
            minlength=T,
        ).astype(np.float32)
        acc += total_gate[:, None] * b2f[None, :]
        full[bb] = acc
    if _trace:
        kernel.last_results = res
    return full
